# revision 59
# baseline (speedup 1.0000x reference)
"""Trainium2 Bass kernel for nn_BaconAdditionReasoner (segment_reduce).

Math (per row b of 1M):
  a = p1 @ minmax(W1); b = p2 @ minmax(W2)           # [10] each
  s_ij = min(a_i, b_j); one_minus = 1 - clip(s)       # [10,10]
  y_k  = 1 - prod_{i+j=k} one_minus_ij                # 19 anti-diag bins
  y    = y / (sum_k y_k + 1e-9)

Kernel formulation:
  alpha = p1 @ (1 - minmax(W1))  (rows of p1 sum to 1) -> one_minus rows
  la_i = ln alpha_i, lb_j = ln beta_j  (fp16, via PE matmul + ACT Ln)
  All per-row tensors are r-innermost ([P, cols, R]): the packed last
  dim is always the row dim, so broadcasts/strides live in middle dims
  and every DVE tensor_tensor keeps the fp16 2x perf mode.
  st[slot(i,j) = 10i+j] = max(la_i, lb_j): the whole 10x10 outer max is
      ONE 2x instruction [P, 10, 10, R]. Anti-diag bin k = {(i, k-i)}
      lives at slots {9i + k}: stride 9, contiguous per bin.
  logP_k = in-place reversed-half fold adds over each bin's slots
      (fp16 2x; mirror bins (c, 18-c) share one instr via a pair dim of
      stride 99-11c), then two batched finals: bins 1..9 read slot pairs
      {k, 9+k}, bins 10..17 read {10k-81, 10k-72} (stride 10).
  y = (1 - exp(logP)) / (19 + 1e-9 - sum exp(logP))

Engine split: PE does per-12-row-block fp16 transposes + matmuls (p1/p2
are converted to fp16 host-side, halving input DMA); ACT does the
batched PSUM->SBUF copies, Ln and Exp (all three funcs pinned to one
activation table to avoid reloads); DVE does all maxes (Pool cannot
encode max), small-class folds, the fp16 4x-mode u = 1-w tensor_scalar
and the normalize scalars; Pool (GPSIMD) takes the big-class fold adds,
the batched final fold, the 19-bin sum L1 and the y = u*r multiply.
Matmul PSUM batches are 4 groups (1920B) so they never cross a 2KB PSUM
bank boundary (bank-crossing matmul windows corrupt on HW even though
the cost model accepts them).

Tile schedule: ramp-up [24,48,80], R=128 mains, ramp-down [64,32,16,8]
to shorten pipeline fill and the serial exp->y->DMA drain.

Sharding: pure data parallel over 8 cores, 131072 rows each.
"""
import sys

if '/opt/trn_rl_repo' not in sys.path:
    sys.path.insert(0, '/opt/trn_rl_repo')

import numpy as np

B = 1048576
N_CORES = 8
RPC = B // N_CORES          # 131072 rows per core
P = 128                     # partitions
NT = 16                     # work units of 8192 rows (for bench scaling)

# class c = mirror bins (c, 18-c), cnt = c+1 slots each; class 9 = bin 9.

# NOTE: the Pool/GPSIMD engine only encodes Add/Subtract/Multiply-type
# tensor_tensor ops in walrus codegen (max is rejected), so maxes stay on
# DVE and Pool takes add/multiply work instead.
POOL_FOLD_CLASSES = (5, 6, 7, 8, 9)  # classes whose fold adds run on Pool
Y_ON_POOL = True            # final y = u*r multiply on Pool vs DVE
FINAL_ON_POOL = True        # batched stride-10 final fold on Pool vs DVE
SUM_L1_ON_POOL = False      # first 19-sum fold level on Pool
R_MAIN = 128                # main tile rows/partition
GROUPS_PER_BATCH = 4        # transpose/matmul groups per PSUM batch


def _groups_for(r):
    """row-group sizes per PE transpose (12 rows of 10 -> K=120)."""
    g = [12] * (r // 12)
    if r % 12:
        g.append(r % 12)
    return g


def _batches_for(r):
    """Batches of uniform-size groups for the PSUM-copy / Ln batching.
    Returns [(row0, gs, ngroups), ...]."""
    out = []
    full = r // 12
    row0 = 0
    while full > 0:
        take = min(GROUPS_PER_BATCH, full)
        out.append((row0, 12, take))
        row0 += 12 * take
        full -= take
    if r % 12:
        out.append((row0, r % 12, 1))
    return out


def _schedule(nt):
    """Small ramp-up tiles (fast pipeline fill), R_MAIN tiles in the
    middle, then small ramp-down tiles so the serial per-tile tail
    (exp -> y -> DMA) drains quickly at the end."""
    rows = nt * P * 64
    lead, trail = [24, 48, 80], [64, 32, 16, 8]
    out, row0 = [], 0
    budget = rows // P
    if budget >= sum(lead) + sum(trail) + R_MAIN:
        for r in lead:
            out.append((row0, r)); row0 += P * r
        budget -= sum(lead) + sum(trail)
    else:
        trail = []
    while budget >= R_MAIN:
        out.append((row0, R_MAIN)); row0 += P * R_MAIN
        budget -= R_MAIN
    if budget > 0:
        out.append((row0, budget)); row0 += P * budget
    for r in trail:
        out.append((row0, r)); row0 += P * r
    assert row0 == rows
    return out

_CACHED = {}


def _build_nc(nt=NT, reps=1):
    import bass_rust as _br
    import concourse.mybir as mybir
    from concourse.bacc import Bacc
    from concourse.mybir import AluOpType
    from concourse.tile import TileContext

    F32 = mybir.dt.float32
    F16 = mybir.dt.float16

    def with_pair(ap_view, pos, stride, n=2):
        raw = ap_view.ap
        raw.insert(pos, [stride, n])
        return _br.AP(tensor=ap_view.tensor, offset=ap_view.offset, ap=raw)

    nc = Bacc()
    p1d = nc.dram_tensor("p1", [RPC, 10], F16, kind="ExternalInput")
    p2d = nc.dram_tensor("p2", [RPC, 10], F16, kind="ExternalInput")
    v1d = nc.dram_tensor("v1b", [120, 120], F16, kind="ExternalInput")
    v2d = nc.dram_tensor("v2b", [120, 120], F16, kind="ExternalInput")
    idd = nc.dram_tensor("ident", [128, 128], F16, kind="ExternalInput")
    yd = nc.dram_tensor("y", [RPC, 19], F32, kind="ExternalOutput")

    sched = _schedule(nt)

    with TileContext(nc) as tc:
        with (
            tc.tile_pool(name="const", bufs=1) as cpool,
            tc.tile_pool(name="io", bufs=4) as io,
            tc.tile_pool(name="pt", bufs=3) as ptp,
            tc.tile_pool(name="ab", bufs=3) as abp,
            tc.tile_pool(name="st", bufs=2) as stp,
            tc.tile_pool(name="lp", bufs=3) as lpp,
            tc.tile_pool(name="w", bufs=3) as wp,
            tc.tile_pool(name="sm", bufs=3) as sm,
            tc.tile_pool(name="tp", bufs=3, space="PSUM") as tpp,
            tc.tile_pool(name="mm", bufs=3, space="PSUM") as mmp,
        ):
            v1t = cpool.tile([120, 120], F16)
            v2t = cpool.tile([120, 120], F16)
            idt = cpool.tile([128, 128], F16)
            nc.sync.dma_start(idt[:], idd[:])
            nc.sync.dma_start(v1t[:], v1d[:])
            nc.sync.dma_start(v2t[:], v2d[:])
            # All ACT funcs used (Ln, Exp, Copy) live in act-table set 6
            # (natural_log_exp_and_others); pin it once instead of letting
            # the auto-pass thrash between the Ln-only and Exp-only sets.
            nc.scalar.add_instruction(mybir.InstLoadActFuncSet(
                name="manual_actload0", act_func_set_id=6))


            for row0, R in [s for _ in range(reps) for s in sched]:
                nrows = P * R
                p1v = p1d[row0:row0 + nrows, :].rearrange(
                    "(p r) c -> p (r c)", p=P)
                p2v = p2d[row0:row0 + nrows, :].rearrange(
                    "(p r) c -> p (r c)", p=P)
                yv = yd[row0:row0 + nrows, :].rearrange(
                    "(p r) k -> p (r k)", p=P)
                p1t = io.tile([P, R * 10], F16, tag="p1t")
                p2t = io.tile([P, R * 10], F16, tag="p2t")
                nc.sync.dma_start(p1t[:], p1v)
                nc.sync.dma_start(p2t[:], p2v)

                # abt is c-major / r-innermost: [P, 20 cols, R]
                abt = abp.tile([P, 20, R], F16, tag="ab")

                for src, vt, o in ((p1t, v1t, 0), (p2t, v2t, 10)):
                    for brow0, gs, ng in _batches_for(R):
                        K = gs * 10
                        tp4 = tpp.tile([128, 128 * GROUPS_PER_BATCH], F16,
                                       tag="tp")
                        mm4 = mmp.tile([P, 120 * GROUPS_PER_BATCH], F32,
                                       tag="mm")
                        for g in range(ng):
                            r0 = brow0 + g * gs
                            nc.tensor.transpose(
                                tp4[0:K, g * 128:(g + 1) * 128],
                                src[:, r0 * 10:(r0 + gs) * 10], idt[:])
                        pt4 = ptp.tile([120, 128 * GROUPS_PER_BATCH], F16,
                                       tag="pt")
                        nc.scalar.copy(pt4[0:K, 0:ng * 128],
                                       tp4[0:K, 0:ng * 128])
                        for g in range(ng):
                            nc.tensor.matmul(
                                mm4[:, g * 120:g * 120 + K],
                                pt4[0:K, g * 128:(g + 1) * 128],
                                vt[0:K, 0:K], start=True, stop=True)
                        # Ln fused into the PSUM->SBUF copy; out is the
                        # [col, row]-strided view of abt
                        ln_in = mm4[:, 0:(ng - 1) * 120 + K].rearrange(
                            "p (g r c) -> p g r c", g=ng, c=10)
                        ln_out = abt[:, o:o + 10,
                                     brow0:brow0 + ng * gs].rearrange(
                            "p c (g r) -> p g r c", g=ng)
                        nc.scalar.activation(
                            ln_out, ln_in,
                            mybir.ActivationFunctionType.Ln)

                # st is slot-major / r-innermost [P, 100, R] with
                # slot(i,j) = 10i+j, so the whole 10x10 outer max is ONE
                # fp16 2x tensor_tensor (broadcasts sit in middle dims,
                # packed r is the last dim). Bin k = {(i, k-i)} lives at
                # slots {9i + k}: stride 9, contiguous per bin.
                st = stp.tile([P, 100, R], F16, tag="st")
                st4 = st[:].rearrange("p (i j) r -> p i j r", j=10)
                a_b = abt[:, 0:10, :].unsqueeze(2).broadcast_to(
                    (P, 10, 10, R))
                b_b = abt[:, 10:20, :].unsqueeze(1).broadcast_to(
                    (P, 10, 10, R))
                nc.vector.tensor_tensor(st4, a_b, b_b, AluOpType.max)

                # in-place reversed-half folds (fp16 2x) down to 2
                # slots/bin; mirror bins (c, 18-c) share one instr via a
                # pair dim of stride (99-11c) slots
                for c in range(2, 10):
                    cnt = c + 1 if c < 9 else 10
                    O = c if c < 9 else 9
                    paired = c < 9
                    pstride = (99 - 11 * c) * R
                    feng = nc.gpsimd if c in POOL_FOLD_CLASSES else nc.vector

                    def pv(s0, h, step):
                        if step > 0:
                            ap = st[:, O + 9 * s0:O + 9 * (s0 + h):9, :]
                        else:
                            ap = st[:, O + 9 * s0:O + 9 * (s0 - h):-9, :]
                        return with_pair(ap, 1, pstride) if paired else ap

                    n = cnt
                    while n > 2:
                        h = n // 2
                        feng.tensor_tensor(pv(0, h, 1), pv(0, h, 1),
                                           pv(n - 1, h, -1), AluOpType.add)
                        n = h + (n & 1)
                # batched finals: bins 1..9 hold partials at slots
                # {k, 9+k}; bins 10..17 at {10k-81, 10k-72} (stride 10);
                # edge bins 0,18 are single-slot copies (4x tensor_scalar)
                lpt = lpp.tile([P, 19, R], F16, tag="lp")
                fin_eng = nc.gpsimd if FINAL_ON_POOL else nc.vector
                fin_eng.tensor_tensor(
                    lpt[:, 1:10, :], st[:, 1:10, :], st[:, 10:19, :],
                    AluOpType.add)
                fin_eng.tensor_tensor(
                    lpt[:, 10:18, :], st[:, 19:90:10, :],
                    st[:, 28:99:10, :], AluOpType.add)
                nc.vector.tensor_scalar(
                    with_pair(lpt[:, 0:1, :], 1, 18 * R),
                    with_pair(st[:, 0:1, :], 1, 99 * R), 1.0, 0.0,
                    AluOpType.mult, AluOpType.add)

                # w = exp(logP) on ACT, fp16 (lpt/wt are k-major)
                wt = wp.tile([P, 19, R], F16, tag="w")
                nc.scalar.activation(
                    wt[:].rearrange("p k r -> p (k r)"),
                    lpt[:].rearrange("p k r -> p (k r)"),
                    mybir.ActivationFunctionType.Exp)
                # u = 1 - w in place first (fp16 packed SBUF
                # tensor_scalar gets the DVE 4x perf mode); then
                # denom = sum(u) + 1e-9 directly, saving the negate op
                nc.vector.tensor_scalar(
                    wt[:].rearrange("p k r -> p (k r)"),
                    wt[:].rearrange("p k r -> p (k r)"),
                    -1.0, 1.0, AluOpType.mult, AluOpType.add)
                # sum(u) via fp16 reversed-half folds into a scratch tile
                # (tensor_reduce has no DVE 2x mode; fold adds do)
                sct = sm.tile([P, 9, R], F16, tag="sc")
                sl1_eng = nc.gpsimd if SUM_L1_ON_POOL else nc.vector
                sl1_eng.tensor_tensor(sct[:], wt[:, 0:9, :],
                                      wt[:, 18:9:-1, :], AluOpType.add)
                n = 9
                while n > 2:
                    h = n // 2
                    nc.vector.tensor_tensor(
                        sct[:, 0:h, :], sct[:, 0:h, :],
                        sct[:, n - 1:n - 1 - h:-1, :], AluOpType.add)
                    n = h + (n & 1)
                # denom = (sct0 + 1e-9) + sct1 + u9; r = 1/denom
                swt = sm.tile([P, R], F32, tag="S")
                nc.vector.scalar_tensor_tensor(
                    swt[:].unsqueeze(1), sct[:, 0:1, :], 1e-9,
                    sct[:, 1:2, :], AluOpType.add, AluOpType.add)
                nc.vector.tensor_tensor(swt[:].unsqueeze(1),
                                        swt[:].unsqueeze(1),
                                        wt[:, 9:10, :], AluOpType.add)
                rt = sm.tile([P, R], F32, tag="r")
                nc.vector.reciprocal(rt[:], swt[:])
                yt = wp.tile([P, R, 19], F32, tag="y")
                r_b = rt[:].unsqueeze(1).broadcast_to((P, 19, R))
                y_eng = nc.gpsimd if Y_ON_POOL else nc.vector
                y_eng.tensor_tensor(yt[:].rearrange("p r k -> p k r"),
                                    wt[:], r_b, AluOpType.mult)
                nc.sync.dma_start(yv, yt[:].rearrange("p r k -> p (r k)"))

    nc.insert_act_table_loads = lambda: None
    nc.finalize()
    return nc


def _host_consts(W1, W2):
    def mmn(W):
        W = W.astype(np.float32)
        lo = W.min(1, keepdims=True)
        hi = W.max(1, keepdims=True)
        return (W - lo) / (hi - lo + np.float32(1e-8))

    eye12 = np.eye(12, dtype=np.float32)
    v1b = np.kron(eye12, (np.float32(1.0) - mmn(W1))).astype(np.float16)
    v2b = np.kron(eye12, (np.float32(1.0) - mmn(W2))).astype(np.float16)
    ident = np.eye(128, dtype=np.float16)
    return v1b, v2b, ident


def kernel(p1, p2, W1, W2, mask=None, **_unused):
    from concourse.bass_utils import run_bass_kernel_spmd

    if 'nc' not in _CACHED:
        _CACHED['nc'] = _build_nc()
    nc = _CACHED['nc']

    v1b, v2b, ident = _host_consts(W1, W2)
    p1 = np.ascontiguousarray(p1, dtype=np.float16)
    p2 = np.ascontiguousarray(p2, dtype=np.float16)

    in_maps = []
    for c in range(N_CORES):
        sl = slice(c * RPC, (c + 1) * RPC)
        in_maps.append({
            "p1": p1[sl], "p2": p2[sl],
            "v1b": v1b, "v2b": v2b, "ident": ident,
        })
    res = run_bass_kernel_spmd(nc, in_maps, list(range(N_CORES)))
    out = np.concatenate([res.results[c]["y"] for c in range(N_CORES)], axis=0)
    return out.astype(np.float32)


if __name__ == "__main__":
    rng = np.random.default_rng(0)
    p1 = rng.random((B, 10), dtype=np.float32)
    p1 /= p1.sum(1, keepdims=True)
    p2 = rng.random((B, 10), dtype=np.float32)
    p2 /= p2.sum(1, keepdims=True)
    W1 = rng.random((10, 10), dtype=np.float32)
    W2 = rng.random((10, 10), dtype=np.float32)
    y = kernel(p1, p2, W1, W2)
    print("kernel ran, y shape", y.shape, "sum", float(y.sum()))
